# revision 21
# baseline (speedup 1.0000x reference)
"""Trainium2 Bass kernel for CustomStaticEdgeConv (GNN message passing).

out[n] = mean_{e: row[e]=n} relu( concat(x[n], x[col_e]-x[n]) @ W.T + b )

Math restructure:
    z_e = P[row_e] + Q[col_e],  P = x@(W1-W2).T + b,  Q = x@W2.T
    relu(z_e) = P + max(Q_e, -P)
    out[n] = P[n]*(1 + pad_n/deg_n) + (1/deg_n) * sum_slots max(Q_slot, -P[n])
(padding slots gather a dummy table row whose MLP output is -1e30, so they
contribute -P[n]; the host folds that into the P coefficient and applies the
1/deg scale itself.)

Device pipeline per core (edges sharded by destination node, 6250 nodes/core):
    dma_gather(transpose=True)  -> x[col] feature-major bf16     [GPSIMD/DMA]
      (striped across 4 SWDGE queues; queue-pairs of Q7 cores overlap ~3x)
    matmul(Baug stationary)     -> Q_T in PSUM fp32              [PE]
    activation copy             -> Q_T bf16 in SBUF              [ACT]
    tensor_tensor(max)          -> M = max(Q, -P) bf16           [DVE]
    tensor_reduce(add, 3D AP)   -> R_T per virtual node          [DVE]
    dma R_T (feature-major)     -> DRAM                          [SYNC/DMA]
Virtual nodes: each node splits by col-half (int16 gather index limit) and is
grouped with equal-degree peers into 128-wide batches so the segmented reduce
is a constant-stride access pattern.
"""

import sys

sys.path.insert(0, "/opt/trn_rl_repo")

import numpy as np
import ml_dtypes

import concourse.bass as bass
import concourse.bacc as bacc
import concourse.mybir as mybir
from concourse.bass_utils import run_bass_kernel_spmd
from concourse.library_config import mlp as mlp_lib

# ---------------------------------------------------------------- constants
N_NODES = 50000
F_IN = 64
F_OUT = 128
N_EDGES = 800000
NCORES = 8
LPC = N_NODES // NCORES  # 6250 nodes per core
CLASS_SPLIT = 32000      # col < 32000 -> lo table, else hi table
# x_pad table layout: [dummy_lo, x[0:32000], dummy_hi, x[32000:50000]]
HI_BASE = CLASS_SPLIT + 1                     # row index of dummy_hi
TAB_ROWS = 2 + N_NODES                        # 50002
DUMMY_CH = F_IN                               # one-hot channel of dummy rows
NEG_BIG = -1.0e30

SEG_SLOTS = 8192         # max slots per dma_gather segment
SUB_SLOTS = 1024         # max slots per PSUM subtile
NQ = 4                   # SWDGE queues (Q7 core pairs)

F32 = mybir.dt.float32
BF16 = mybir.dt.bfloat16
I16 = mybir.dt.int16


# ---------------------------------------------------------------- host prep
def _plan_and_pack(edge_index):
    """Build the shared SPMD batch plan and per-core index blobs.

    Returns (plan, per_core) where plan is identical across cores
    (drives codegen) and per_core holds DRAM inputs + assembly metadata.
    """
    rows = np.asarray(edge_index[0], dtype=np.int64)
    cols = np.asarray(edge_index[1], dtype=np.int64)
    core = rows // LPC
    loc_row = (rows - core * LPC).astype(np.int32)
    cls = (cols >= CLASS_SPLIT).astype(np.int32)
    # gather index within class table (dummy row of each class is index 0)
    gidx = np.where(cls == 0, cols + 1, cols - CLASS_SPLIT + 1).astype(np.int32)

    # order edges by (core, class, local_row) -> virtual nodes are runs
    order = np.lexsort((loc_row, cls, core))
    core_s, cls_s, lr_s, gi_s = core[order], cls[order], loc_row[order], gidx[order]

    cores = []
    for c in range(NCORES):
        sel = core_s == c
        cc, ll, gg = cls_s[sel], lr_s[sel], gi_s[sel]
        # virtual node = unique (class, local_row) run
        key = cc.astype(np.int64) * LPC + ll
        ukey, start, vdeg = np.unique(key, return_index=True, return_counts=True)
        vcls = (ukey // LPC).astype(np.int32)
        vnode = (ukey % LPC).astype(np.int32)
        # true degree per local node
        deg = np.bincount(ll, minlength=LPC).astype(np.int64)
        cores.append(dict(cc=cc, ll=ll, gg=gg, start=start, vdeg=vdeg.astype(np.int64),
                          vcls=vcls, vnode=vnode, deg=deg))

    # --- shared batch plan: per class, batches of 128 virtuals sorted by deg desc
    plan_batches = []  # list of (cls, g)
    for h in (0, 1):
        per_core_sorted = []
        for c in range(NCORES):
            d = cores[c]
            m = d["vcls"] == h
            sd = np.sort(d["vdeg"][m])[::-1]
            per_core_sorted.append(sd)
        nb = max((len(s) + 127) // 128 for s in per_core_sorted)
        for j in range(nb):
            g = 1
            for s in per_core_sorted:
                if len(s) > j * 128:
                    g = max(g, int(s[j * 128]))
            g = (g + 1) & ~1  # even: the DVE max op's inner contiguous pair
            plan_batches.append((h, g))  # is its fast path (0.71 vs 1.2 ns/col)

    nbatch = len(plan_batches)
    tot_slots = sum(128 * g for (_h, g) in plan_batches)
    assert tot_slots % 16 == 0

    # --- segments: runs of same-class batches, <= SEG_SLOTS slots each.
    # The first NQ*2 segments are half-size: desc-gen (one queue pair each,
    # ~7.7ns/idx) must stay ahead of the serialized gather DMA stream
    # (~3ns/idx), so the ramp keeps early triggers from stalling on gen.
    segments = []  # (cls, slot_start, nslots)
    s_start, s_cls, s_n = 0, plan_batches[0][0], 0
    off = 0
    for (h, g) in plan_batches:
        bs = 128 * g
        cap = SEG_SLOTS // 2 if len(segments) < NQ else SEG_SLOTS
        if h != s_cls or s_n + bs > cap:
            segments.append((s_cls, s_start, s_n))
            s_start, s_cls, s_n = off, h, 0
        s_n += bs
        off += bs
    segments.append((s_cls, s_start, s_n))

    # --- subtiles: (batch, node offset in batch, n_sub, slot offset) global
    subtiles = []
    off = 0
    cum_sub = []  # number of subtiles after each batch
    for bj, (h, g) in enumerate(plan_batches):
        done = 0
        while done < 128:
            n_sub = min(128 - done, SUB_SLOTS // g)
            subtiles.append(dict(batch=bj, n0=done, n_sub=n_sub,
                                 slot=off + done * g, g=g))
            done += n_sub
        off += 128 * g
        cum_sub.append(len(subtiles))
    # attach segment id to each subtile
    seg_of_slot = np.zeros(tot_slots + 1, dtype=np.int64)
    for si, (_h, st, ns) in enumerate(segments):
        seg_of_slot[st:st + ns] = si
    for t in subtiles:
        t["seg"] = int(seg_of_slot[t["slot"]])

    plan = dict(batches=plan_batches, segments=segments, subtiles=subtiles,
                nbatch=nbatch, tot_slots=tot_slots, cum_sub=cum_sub)

    # --- per-core packing
    per_core = []
    for c in range(NCORES):
        d = cores[c]
        nv = len(d["vdeg"])
        # sort this core's virtuals into plan order: class, then deg desc
        vorder = np.lexsort((-d["vdeg"], d["vcls"]))
        # per-class partition points in plan batches
        slot_blob = np.zeros(tot_slots, dtype=np.int16)
        vmap_node = np.full(nbatch * 128, -1, dtype=np.int64)  # virtual -> local node
        pad_per_node = np.zeros(LPC, dtype=np.int64)

        # iterate plan batches, consuming this core's sorted virtuals per class
        ptr = {0: 0, 1: 0}
        cls_sorted = {h: vorder[d["vcls"][vorder] == h] for h in (0, 1)}
        off = 0
        for bj, (h, g) in enumerate(plan_batches):
            lst = cls_sorted[h]
            take = lst[ptr[h]:ptr[h] + 128]
            ptr[h] += len(take)
            for p, vi in enumerate(take):
                dg = int(d["vdeg"][vi])
                st = int(d["start"][vi])
                assert dg <= g
                sl = off + p * g
                slot_blob[sl:sl + dg] = d["gg"][st:st + dg].astype(np.int16)
                # remaining g-dg slots stay 0 (dummy row of the class table)
                node = int(d["vnode"][vi])
                vmap_node[bj * 128 + p] = node
                pad_per_node[node] += g - dg
            off += 128 * g

        # wrapped idx layout for dma_gather: w[p, ccol] = blob[ccol*16 + p%16]
        wrapped = np.tile(slot_blob.reshape(-1, 16).T, (8, 1)).astype(np.int16)

        # per-virtual x (permuted, duplicated per virtual), feature-major +ones
        lpadv = nbatch * 128
        xpt = np.zeros((F_IN + 1, lpadv), dtype=np.float32)
        per_core.append(dict(wrapped=wrapped, xpt=xpt,
                             vmap_node=vmap_node, pad_per_node=pad_per_node,
                             deg=d["deg"], lpadv=lpadv))
    return plan, per_core


def _build_program(plan):
    nbatch = plan["nbatch"]
    tot = plan["tot_slots"]
    segs = plan["segments"]
    subs = plan["subtiles"]
    lpadv = nbatch * 128
    n_pchunk = (lpadv + 511) // 512

    # 32KB descriptor carveout: a 8192-idx gather preps ~514 descs/engine;
    # the default 16KB ring stalls prepare_only in await_space when two
    # segments of one queue are in flight, cascading into gp-engine stalls.
    nc = bacc.Bacc("TRN2", num_swdge_queues=NQ, dynamic_dma_scratch_size=32768)
    xpad_d = nc.dram_tensor("xpad", [TAB_ROWS, 2 * F_IN], BF16, kind="ExternalInput")
    xpt_d = nc.dram_tensor("xpt", [F_IN + 1, lpadv], BF16, kind="ExternalInput")
    aaug_d = nc.dram_tensor("aaug", [F_IN + 1, F_OUT], BF16, kind="ExternalInput")
    baug_d = nc.dram_tensor("baug", [2 * F_IN, F_OUT], BF16, kind="ExternalInput")
    idx_d = nc.dram_tensor("idx", [128, tot // 16], I16, kind="ExternalInput")
    sout_d = nc.dram_tensor("sout", [F_OUT, lpadv], F32, kind="ExternalOutput")
    pout_d = nc.dram_tensor("pout", [F_OUT, lpadv], F32, kind="ExternalOutput")

    from contextlib import ExitStack

    with ExitStack() as ctx:
        block = ctx.enter_context(nc.Block())
        sb = lambda name, shape, dt: ctx.enter_context(nc.sbuf_tensor(name, shape, dt))
        ps = lambda name, shape: ctx.enter_context(nc.psum_tensor(name, shape, F32))
        sem = lambda name: ctx.enter_context(nc.semaphore(name))

        xg = [sb(f"xg{i}", [128, SEG_SLOTS], BF16) for i in range(NQ)]
        idxs = sb("idxs", [128, tot // 16], I16)
        np2 = sb("np2", [128, 2 * lpadv], BF16)        # -P, col pairs
        xpt_s = sb("xpt_s", [F_IN + 1, lpadv], BF16)
        qs0 = sb("qs0", [128, SUB_SLOTS], BF16)        # Q bf16 drain
        qs1 = sb("qs1", [128, SUB_SLOTS], BF16)
        m0 = sb("m0", [128, SUB_SLOTS], BF16)
        m1 = sb("m1", [128, SUB_SLOTS], BF16)
        rt0 = sb("rt0", [128, 128], F32)
        rt1 = sb("rt1", [128, 128], F32)
        ptc0 = sb("ptc0", [128, 512], F32)
        ptc1 = sb("ptc1", [128, 512], F32)
        aaug_s = sb("aaug_s", [F_IN + 1, F_OUT], BF16)
        baug_s = sb("baug_s", [2 * F_IN, F_OUT], BF16)
        pq0 = ps("pq0", [128, SUB_SLOTS])
        pq1 = ps("pq1", [128, SUB_SLOTS])
        pp0 = ps("pp0", [128, 512])
        pp1 = ps("pp1", [128, 512])
        s_in = sem("s_in")
        s_idx = sem("s_idx")
        s_gd = sem("s_gd")
        s_prep = sem("s_prep")
        s_mm = sem("s_mm")
        s_pp = sem("s_pp")
        s_ptd = sem("s_ptd")
        s_np = sem("s_np")
        s_qd = sem("s_qd")
        s_tt = sem("s_tt")
        s_red = sem("s_red")
        s_out = [sem("s_out0"), sem("s_out1")]
        s_pto = [sem("s_pto0"), sem("s_pto1")]
        qs = [qs0, qs1]
        m = [m0, m1]
        rt = [rt0, rt1]
        ptc = [ptc0, ptc1]
        pq = [pq0, pq1]
        pp = [pp0, pp1]

        nseg = len(segs)
        nsub = len(subs)
        N_IN_DMAS = 3  # xpt, aaug, baug

        # last subtile index per segment (for gather buffer recycling)
        last_sub_of_seg = {}
        for t_i, t in enumerate(subs):
            last_sub_of_seg[t["seg"]] = t_i

        @block.sync
        def _(sync):
            sync.dma_start(idxs[:, :], idx_d[:, :]).then_inc(s_idx, 16)
            sync.dma_start(xpt_s[:, :], xpt_d[:, :]).then_inc(s_in, 16)
            sync.dma_start(aaug_s[:, :], aaug_d[:, :]).then_inc(s_in, 16)
            sync.dma_start(baug_s[:, :], baug_d[:, :]).then_inc(s_in, 16)
            # P out, chunk by chunk (after ACT drains it)
            for k in range(n_pchunk):
                w = min(512, lpadv - 512 * k)
                sync.wait_ge(s_ptd, k + 1)
                sync.dma_start(pout_d[:, 512 * k:512 * k + w],
                               ptc[k % 2][:, :w]).then_inc(s_pto[k % 2], 16)
            # R out, feature-major, batch by batch after DVE reduce
            for j in range(nbatch):
                sync.wait_ge(s_red, plan["cum_sub"][j])
                sync.dma_start(sout_d[:, 128 * j:128 * (j + 1)],
                               rt[j % 2][:, :]).then_inc(s_out[j % 2], 16)

        @block.gpsimd
        def _(gp):
            # Descriptor generation overlaps across the NQ Q7 queue pairs
            # (prepare_only); DMA execution is strictly serialized via
            # trigger_dma because concurrent transpose-gather rx streams
            # corrupt each other in the shared xbar staging.
            gp.load_library(mlp_lib)
            gp.wait_ge(s_idx, 16)

            def prep(si):
                h, st, ns = segs[si]
                base = 0 if h == 0 else HI_BASE
                nrows = (HI_BASE if h == 0 else TAB_ROWS) - base
                gp.dma_gather(
                    xg[si % NQ][:, :ns].rearrange("p (a s) -> p a s", a=1),
                    xpad_d[base:base + nrows, :],
                    idxs[:, st // 16:(st + ns) // 16],
                    ns, ns, 2 * F_IN,
                    transpose=True,
                    single_packet=False,
                    queue_num=si % NQ,
                    prepare_only=True,
                    sem=s_gd,
                ).then_inc(s_prep, 1)

            def trig(si):
                gp.wait_ge(s_prep, si + 1)
                if si >= 1:
                    gp.wait_ge(s_gd, 16 * si)  # previous segment's DMA done
                if si >= NQ:
                    # xg[si%NQ] free only after PE consumed segment si-NQ
                    gp.wait_ge(s_mm, last_sub_of_seg[si - NQ] + 1)
                gp.trigger_dma(count=1, queue_num=si % NQ)

            # order: p0..p3 [t0 p4] [t1 p5] ... [t_j p_{j+4}] ... t_{n-1}
            # (p_{j+4} reuses queue j%NQ, free once t_j's wait on p_j resolved;
            # the 32KB ring holds both segments' descriptors)
            for si in range(min(NQ, nseg)):
                prep(si)
            for si in range(nseg):
                trig(si)
                if si + NQ < nseg:
                    prep(si + NQ)

        @block.tensor
        def _(pe):
            pe.wait_ge(s_in, 16 * N_IN_DMAS)
            # P_T = Aaug.T @ xpt  (per-virtual P, feature-major)
            for k in range(n_pchunk):
                w = min(512, lpadv - 512 * k)
                if k >= 2:
                    pe.wait_ge(s_np, k - 1)  # pp[k%2] free after DVE consumed it
                pe.matmul(pp[k % 2][:, :w], aaug_s[:, :],
                          xpt_s[:, 512 * k:512 * k + w],
                          start=True, stop=True).then_inc(s_pp)
            # main loop: MLP matmuls
            for t_i, t in enumerate(subs):
                ncols = t["n_sub"] * t["g"]
                sg = t["seg"]
                pe.wait_ge(s_gd, 16 * (sg + 1))
                if t_i >= 2:
                    pe.wait_ge(s_qd, t_i - 1)  # pq[t_i%2] free after ACT drain
                soff = t["slot"] - segs[sg][1]
                # one matmul per PSUM bank (max 512 fp32 output columns)
                for c0 in range(0, ncols, 512):
                    w = min(512, ncols - c0)
                    mm = pe.matmul(pq[t_i % 2][:, c0:c0 + w], baug_s[:, :],
                                   xg[sg % NQ][:, soff + c0:soff + c0 + w],
                                   start=True, stop=True)
                    if c0 + w == ncols:
                        mm.then_inc(s_mm)

        @block.scalar
        def _(act):
            # P_T drain: PSUM -> SBUF chunks (also feeds DVE negP build + DMA out)
            for k in range(n_pchunk):
                w = min(512, lpadv - 512 * k)
                act.wait_ge(s_pp, k + 1)
                if k >= 2:
                    act.wait_ge(s_pto[k % 2], 16 * (k // 2))  # ptc[k%2] free
                act.activation(ptc[k % 2][:, :w], pp[k % 2][:, :w],
                               mybir.ActivationFunctionType.Copy).then_inc(s_ptd)
            # Q drain: PSUM fp32 -> SBUF bf16
            for t_i, t in enumerate(subs):
                ncols = t["n_sub"] * t["g"]
                act.wait_ge(s_mm, t_i + 1)
                if t_i >= 2:
                    act.wait_ge(s_tt, t_i - 1)  # qs[t_i%2] free after DVE max
                act.activation(qs[t_i % 2][:, :ncols], pq[t_i % 2][:, :ncols],
                               mybir.ActivationFunctionType.Copy).then_inc(s_qd)

        @block.vector
        def _(dve):
            # negP build: pp PSUM -> -P bf16
            for k in range(n_pchunk):
                w = min(512, lpadv - 512 * k)
                dve.wait_ge(s_ptd, k + 1)  # after ACT drained (pp stable, and
                # ordering with PE reuse is via s_np waits on PE side)
                dve.tensor_scalar_mul(
                    np2[:, 1024 * k:1024 * k + 2 * w].rearrange("p (n two) -> p n two", two=2),
                    pp[k % 2][:, :w].rearrange("p (n one) -> p n one", one=1)
                        .to_broadcast([128, w, 2]),
                    -1.0,
                ).then_inc(s_np)
            # max + grouped reduce, software-pipelined by one subtile
            def emit_reduce(t_i):
                t = subs[t_i]
                g = t["g"]
                bj = t["batch"]
                dve.wait_ge(s_tt, t_i + 1)  # own max op retired (deep pipeline)
                if bj >= 2 and t["n0"] == 0:
                    dve.wait_ge(s_out[bj % 2], 16 * (bj // 2))  # rt[bj%2] free
                dve.tensor_reduce(
                    rt[bj % 2][:, t["n0"]:t["n0"] + t["n_sub"]],
                    m[t_i % 2][:, :t["n_sub"] * g].rearrange("p (n g) -> p n g", g=g),
                    axis=mybir.AxisListType.X,
                    op=mybir.AluOpType.add,
                ).then_inc(s_red)

            for t_i, t in enumerate(subs):
                g = t["g"]
                ncols = t["n_sub"] * g
                n0 = t["batch"] * 128 + t["n0"]
                dve.wait_ge(s_qd, t_i + 1)
                if t_i == 0:
                    dve.wait_ge(s_np, n_pchunk)
                if t_i >= 2:
                    dve.wait_ge(s_red, t_i - 1)  # m[t_i%2] free
                dve.tensor_tensor(
                    m[t_i % 2][:, :ncols].rearrange("p (n h two) -> p n h two", h=g // 2, two=2),
                    qs[t_i % 2][:, :ncols].rearrange("p (n h two) -> p n h two", h=g // 2, two=2),
                    np2[:, 2 * n0:2 * (n0 + t["n_sub"])]
                        .rearrange("p (n one two) -> p n one two", one=1, two=2)
                        .to_broadcast([128, t["n_sub"], g // 2, 2]),
                    op=mybir.AluOpType.max,
                ).then_inc(s_tt)
                if t_i >= 1:
                    emit_reduce(t_i - 1)
            emit_reduce(nsub - 1)

    nc.compile()
    return nc


_CACHE = {}
TRACE = False
LAST_EXEC_NS = None


def kernel(x, edge_index, W, b):
    x = np.asarray(x, dtype=np.float32)
    W = np.asarray(W, dtype=np.float32)
    b = np.asarray(b, dtype=np.float32)
    plan, per_core = _plan_and_pack(edge_index)

    key = (plan["tot_slots"], plan["nbatch"], tuple(plan["batches"]))
    if key not in _CACHE:
        _CACHE[key] = _build_program(plan)
    nc = _CACHE[key]

    # ---- global tables
    W1, W2 = W[:, :F_IN], W[:, F_IN:]
    A = (W1 - W2).T.astype(np.float32)          # [64, 128]
    B = W2.T.astype(np.float32)                 # [64, 128]
    aaug = np.concatenate([A, b[None, :]], axis=0).astype(ml_dtypes.bfloat16)
    baug = np.zeros((2 * F_IN, F_OUT), dtype=np.float32)
    baug[:F_IN] = B
    baug[DUMMY_CH, :] = NEG_BIG
    baug = baug.astype(ml_dtypes.bfloat16)

    xpad = np.zeros((TAB_ROWS, 2 * F_IN), dtype=ml_dtypes.bfloat16)
    xb = x.astype(ml_dtypes.bfloat16)
    xpad[1:1 + CLASS_SPLIT, :F_IN] = xb[:CLASS_SPLIT]
    xpad[HI_BASE + 1:HI_BASE + 1 + (N_NODES - CLASS_SPLIT), :F_IN] = xb[CLASS_SPLIT:]
    xpad[0, DUMMY_CH] = 1.0
    xpad[HI_BASE, DUMMY_CH] = 1.0

    in_maps = []
    for c in range(NCORES):
        pc = per_core[c]
        # per-virtual x columns (feature-major, ones row for bias)
        vmap = pc["vmap_node"]
        xpt = pc["xpt"]
        valid = vmap >= 0
        gl = np.zeros(len(vmap), dtype=np.int64)
        gl[valid] = vmap[valid] + c * LPC
        xpt[:F_IN, :] = np.where(valid[None, :], x[gl].T, 0.0)
        xpt[F_IN, :] = np.where(valid, 1.0, 0.0)
        in_maps.append({
            "xpad": xpad, "xpt": xpt.astype(ml_dtypes.bfloat16),
            "aaug": aaug, "baug": baug,
            "idx": pc["wrapped"],
        })

    global LAST_EXEC_NS
    res = run_bass_kernel_spmd(nc, in_maps, core_ids=list(range(NCORES)),
                               trace=TRACE)
    if TRACE:
        LAST_EXEC_NS = res.exec_time_ns

    # ---- assembly
    out = np.zeros((N_NODES, F_OUT), dtype=np.float32)
    for c in range(NCORES):
        pc = per_core[c]
        RT = res.results[c]["sout"]         # [128, lpadv] = R (unscaled) per virtual
        PT = res.results[c]["pout"]         # [128, lpadv] = P per virtual
        vmap = pc["vmap_node"]
        valid = vmap >= 0
        deg = pc["deg"]                     # true degree per local node
        pad = pc["pad_per_node"]
        acc = np.zeros((LPC, F_OUT), dtype=np.float32)
        np.add.at(acc, vmap[valid], RT.T[valid])
        # P per local node (first virtual of each node carries it)
        P_loc = np.zeros((LPC, F_OUT), dtype=np.float32)
        P_loc[vmap[valid]] = PT.T[valid]
        invdeg = (1.0 / np.maximum(deg, 1))[:, None].astype(np.float32)
        c1 = (1.0 + pad[:, None] * invdeg).astype(np.float32)
        loc = P_loc * c1 + acc * invdeg
        loc[deg == 0] = 0.0
        out[c * LPC:(c + 1) * LPC] = loc
    return out
